# revision 24
# baseline (speedup 1.0000x reference)
"""Trainium2 Bass kernel for nn_Attention_82660940579436.

Computation (see reference):
    q     = mean_s(hidden @ Wq.T + bq)            [B, H]
    key   = tanh(hidden @ Wk.T + bk)              [S, B, H]
    score = einsum('bsh,bh->bs', key, q) + mask   [B, S]
    out   = softmax(score) @ key                  [B, H]

Sharding: data-parallel over batch. B=32 over 8 cores -> 4 batches/core.

Fused streaming design (single pass, software-pipelined emission):
  - h tiles [128 tok=(s,g), 512 j] stream in via SWDGE (fp32->bf16 cast),
    8 tiles per DMA, 12-slab ring.
  - q-sum runs as tiny rank-4 PE matmuls on the UNtransposed h tiles
    (out [128 j, (c,g)] accumulated in PSUM over all tiles) -- q is ready
    right after the last load (~48us), not after the last key matmul.
  - hT: first NX tiles via PE transpose + DVE copy; the rest via the DMA
    XBAR transpose (SBUF->SBUF, 2 tiles per instruction). The xbar emits
    a fixed token permutation (position m holds token 2*(m%64)+m//64 of
    its tile); the permutation is absorbed into the constant packs (mask
    columns, ind4, qrep) -- every downstream op is token-parallel or a
    permutation-invariant contraction over tokens.
  - key matmul: bf16, moving operand = Wk chunks (4 x 512 cols); bias via
    a single fp8e4 DoubleRow matmul (K=1, 0.5 cycles/col, exact for these
    operands -- verified on HW).
  - tanh on ACT over 2-tile PSUM batches -> keys ring.
  - score pipeline trails by LAG tiles (the q barrier): prod on DVE/Pool,
    row-reduce on DVE (2-tile) / ACT (copy+accum), exp with the mask as a
    per-partition bias, ei = ind4*e (DVE TSP), numer/den as rank-4/1 PE
    matmuls accumulated in PSUM ([128 i, (c,g)] resp. [1, 4]).
  - epilogue: rcp(den), replicate via tiny matmul, scale, DMA out
    [128, (c,g)]; host reorders to [B, H].

fp8 for the big matmul was measured (numpy) at rel_err 3.9e-2 -- over the
2e-2 gate -- so the key matmul stays bf16.
"""

import sys
from contextlib import ExitStack

import numpy as np

if "/opt/trn_rl_repo" not in sys.path:
    sys.path.insert(0, "/opt/trn_rl_repo")

import ml_dtypes  # noqa: E402

import concourse.bacc as bacc  # noqa: E402
import concourse.mybir as mybir  # noqa: E402
import concourse.tile as tile  # noqa: E402
from concourse.bass_utils import run_bass_kernel_spmd  # noqa: E402

S, B, H = 4096, 32, 512
NCORES = 8
BPC = B // NCORES  # 4 batches per core
NT = 128  # tiles per core
SS = S // NT  # 32 s-positions per tile
TOK = SS * BPC  # 128 tokens per tile
HC = H // 128  # 4 chunks of the H (j / i) dims
MASK_NEG = -60.0
F32 = mybir.dt.float32
BF16 = mybir.dt.bfloat16
FP8 = mybir.dt.float8e4
AF = mybir.ActivationFunctionType
ALU = mybir.AluOpType
PM = mybir.MatmulPerfMode
BF16NP = ml_dtypes.bfloat16
FP8NP = ml_dtypes.float8_e4m3

# ---- tuning knobs ----
KNOBS = {
    "NX": 48,  # tiles using PE transpose; rest use xbar DMA transpose
    "LAG": 44,  # score pipeline trails key pipeline by this many tiles
    "KQ": 45,  # emit q computation after this key tile
    "H_BUFS": 24,  # h ring slabs (LOADS_PER_DMA tiles each)
    "K_BUFS": 28,  # keys ring (2 tiles each)
    "HT_BUFS": 1,  # hT sbuf bufs (PE-transposed tiles)
    "HTX_BUFS": 4,  # hT sbuf bufs (xbar pairs)
    "PROD_DVE": (11, 20),  # prod on DVE for t%20 < 11, else gpsimd
    "RED_DVE": (7, 10),  # reduce on DVE for pair%10 < 7, else ACT
    "BIAS_FP8": False,  # DR matmuls corrupt interleaved PSUM accumulation on HW
    "LOADS_PER_DMA": 4,
    "DEBUG": False,
}

# fp32 const pack layout (offsets in fp32 elements, [128, PACKF] tensor)
OFF_MASK = 0  # [128, NT] mask bias (0 / MASK_NEG), col=tile (perm-aware)
OFF_BQ = NT  # [4, 512] bq rows
OFF_ONEROW_F = NT + 512  # [1, 128] ones (fp32) for rcp_rep matmul
OFF_ZERO = NT + 640  # [128, 1] zeros (tanh bias)
PACKF = NT + 641

# bf16 const pack layout ([128, PACKB])
OFFB_WK = 0  # [128, 2048] WkT chunks
OFFB_WQ = 2048  # [128, 2048] (WqT/S) chunks
OFFB_ID = 4096  # [128, 128] identity
OFFB_I4_NAT = 4224  # [128, 4] indicator
OFFB_I4_PERM = 4228  # [128, 4] indicator (xbar permuted)
OFFB_ONES1 = 4232  # [128, 1] ones
OFFB_ZROW = 4233  # [1, 23] zeros (zero-init matmuls)
OFFB_BKROW = 4256  # [1, 512] bk (bf16 bias fallback)
OFFB_ONEROW = 4768  # [1, 128] ones row
OFFB_I4T_NAT = 4896  # [4, 128] indicator transposed (bf16)
OFFB_I4T_PERM = 5024  # [4, 128] indicator transposed, permuted (bf16)
PACKB = 5152

# fp8 pack ([1, PACK8]): DoubleRow bias operands
OFF8_L = 0  # [1, 256] lhsT pairs: slot0 = ones(128), slot1 = zeros
OFF8_R = 256  # [1, 1024] rhs pairs: slot0 = bk, slot1 = zeros
PACK8 = 1280


def _xbar_perm():
    """Token permutation of the xbar output: position m holds token 2*(m%64)+m//64."""
    m = np.arange(128)
    return 2 * (m % 64) + m // 64


def _build_kernel_body(tc, aps):
    nc = tc.nc
    x, packf, packb, pack8 = aps["x"], aps["packf"], aps["packb"], aps["pack8"]
    y_num = aps["y_num"]
    dbg = KNOBS["DEBUG"]

    NX = KNOBS["NX"]
    LAG = KNOBS["LAG"]
    KQ = KNOBS["KQ"]
    LPD = KNOBS["LOADS_PER_DMA"]
    NB = NT // LPD  # load batches
    HB = KNOBS["H_BUFS"]

    with ExitStack() as ctx:
        consts = ctx.enter_context(tc.tile_pool(name="consts", bufs=1))
        ph = ctx.enter_context(tc.tile_pool(name="h", bufs=HB))
        phT = ctx.enter_context(tc.tile_pool(name="hT", bufs=KNOBS["HT_BUFS"]))
        phTx = ctx.enter_context(tc.tile_pool(name="hTx", bufs=KNOBS["HTX_BUFS"]))
        pkeys = ctx.enter_context(tc.tile_pool(name="keys", bufs=KNOBS["K_BUFS"]))
        pprod = ctx.enter_context(tc.tile_pool(name="prod", bufs=4))
        psmall = ctx.enter_context(tc.tile_pool(name="small", bufs=6))
        pout = ctx.enter_context(tc.tile_pool(name="out", bufs=1))
        pps_q = ctx.enter_context(tc.tile_pool(name="ps_q", bufs=1, space="PSUM"))
        pps_key = ctx.enter_context(tc.tile_pool(name="ps_key", bufs=2, space="PSUM"))
        pps_hT = ctx.enter_context(tc.tile_pool(name="ps_hT", bufs=2, space="PSUM"))
        pps_d = ctx.enter_context(tc.tile_pool(name="ps_d", bufs=1, space="PSUM"))

        # ---- constants: one DMA per pack ----
        cf = consts.tile([128, PACKF], F32)
        nc.sync.dma_start(cf, packf)
        cb = consts.tile([128, PACKB], BF16)
        nc.sync.dma_start(cb, packb)
        c8 = consts.tile([1, PACK8], FP8)
        nc.sync.dma_start(c8, pack8)

        def wk_sb(c):
            return cb[:, OFFB_WK + c * 512 : OFFB_WK + (c + 1) * 512]

        def wq_sb(c):
            return cb[:, OFFB_WQ + c * 512 : OFFB_WQ + (c + 1) * 512]

        id_sb = cb[:, OFFB_ID : OFFB_ID + 128]
        maskb_sb = cf[:, OFF_MASK : OFF_MASK + NT]
        bq_sb = cf[0:BPC, OFF_BQ : OFF_BQ + H]
        onerow_f = cf[0:1, OFF_ONEROW_F : OFF_ONEROW_F + 128]
        zero_sb = cf[:, OFF_ZERO : OFF_ZERO + 1]
        i4_nat = cb[:, OFFB_I4_NAT : OFFB_I4_NAT + BPC]
        i4_perm = cb[:, OFFB_I4_PERM : OFFB_I4_PERM + BPC]
        ones1 = cb[:, OFFB_ONES1 : OFFB_ONES1 + 1]
        zrow = cb[0:1, OFFB_ZROW : OFFB_ZROW + 23]
        bkrow = cb[0:1, OFFB_BKROW : OFFB_BKROW + H]
        onerow_b = cb[0:1, OFFB_ONEROW : OFFB_ONEROW + 128]
        i4t_nat = cb[0:BPC, OFFB_I4T_NAT : OFFB_I4T_NAT + 128]
        i4t_perm = cb[0:BPC, OFFB_I4T_PERM : OFFB_I4T_PERM + 128]
        bias8_l = c8[:, OFF8_L : OFF8_L + 256].rearrange(
            "p (two m) -> p two m", two=2
        )
        bias8_r = c8[:, OFF8_R : OFF8_R + 1024].rearrange(
            "p (two n) -> p two n", two=2
        )

        # Shared PSUM banks: Tq (dummies then qacc), Td (q/qrep/den/rr chain).
        Tq = pps_q.tile([128, 512], F32, tag="qa")
        Td = pps_d.tile([128, 512], F32, tag="d")

        # Dummy PE ops observing each const-pack DMA lane once (walrus allows
        # only ONE sync-wait per Matmult).
        nc.tensor.matmul(
            Tq, bias8_l, bias8_r, start=True, stop=True, perf_mode=PM.DoubleRow
        )
        nc.tensor.matmul(
            Tq[:, 0:128], onerow_b, onerow_b, start=True, stop=True
        )
        nc.tensor.matmul(
            Tq[:, 128:256], onerow_f, cf[0:1, 0:128], start=True, stop=True
        )

        # qacc: [128 j_local, (c, g)] accumulated over all tiles; zero-init
        # matmul so per-tile qsum matmuls never need start=True.
        qacc_ps = Tq[:, 0 : HC * BPC]
        nc.tensor.matmul(
            qacc_ps, onerow_b, zrow[:, 0:16], start=True, stop=False,
            skip_group_check=True,
        )

        h_slabs = [None] * HB
        hT_nat = [None] * NT  # per-tile [128, 512] (PE transpose path)
        hTx_pairs = [None] * (NT // 2)  # per-pair [128, 1024] (xbar path)
        key_pairs = [None] * (NT // 2)
        pc_tile = pout.tile([TOK, H], BF16, tag="pc")  # ACT-reduce dump
        pair_bufs = {}
        if dbg:
            e_all = pout.tile([TOK, NT], F32, tag="e_all")
        else:
            e_all = None
        state = {
            "q_done": False,
            "qrep_nat": None,
            "qrep_perm": None,
            "numer_ps": None,
            "den_ps": None,
            "prod_pair": None,
            "sc_pair": None,
            "next_load": min(HB, NB),
            "s_prod": 0,
            "s_red": 0,
            "s_post": 0,
        }

        def h_tile(t):
            return h_slabs[(t // LPD) % HB][:, (t % LPD) * H : (t % LPD + 1) * H]

        def emit_load(b):
            slab = ph.tile([TOK, LPD * H], BF16, tag="h")
            h_slabs[b % HB] = slab
            nc.gpsimd.dma_start(
                slab, x[b * LPD : (b + 1) * LPD].rearrange("t p j -> p t j")
            )

        def emit_qsum(t):
            ht = h_tile(t)
            for c in range(HC):
                nc.tensor.matmul(
                    qacc_ps[:, c * BPC : (c + 1) * BPC],
                    ht[:, c * 128 : (c + 1) * 128],
                    i4_nat,
                    start=False,
                    stop=(t == NT - 1 and c == HC - 1),
                    skip_group_check=True,
                )

        def emit_transpose(t):
            hT_ps = pps_hT.tile([128, H], BF16, tag="hT")
            ht = h_tile(t)
            for c in range(HC):
                nc.tensor.transpose(
                    hT_ps[:, c * 128 : (c + 1) * 128],
                    ht[:, c * 128 : (c + 1) * 128],
                    id_sb,
                )
            hT_sb = phT.tile([128, H], BF16, tag="hT_sb")
            nc.vector.tensor_copy(hT_sb, hT_ps)
            hT_nat[t] = hT_sb

        def emit_xbar(t):
            # XBAR transpose of the pair (t, t+1). With a 3D out AP
            # [p, cb (stride 128), k (stride 1)] the xbar lands NATURALLY:
            # hTx[j, cb*128 + tok] = h[tok, cb*128 + j], cb = tp*4 + c.
            slab = h_slabs[(t // LPD) % HB]
            lo = (t % LPD) * H
            hTx = phTx.tile([128, 2 * H], BF16, tag="hTx")
            nc.sync.dma_start(
                hTx.rearrange("p (cb k) -> p cb k", k=128),
                slab[:, lo : lo + 2 * H],
                transpose=True,
            )
            hTx_pairs[t // 2] = hTx

        def lhsT_for(t, c):
            if t < NX:
                return hT_nat[t][:, c * 128 : (c + 1) * 128]
            hTx = hTx_pairs[t // 2]
            cb = (t % 2) * HC + c
            return hTx[:, cb * 128 : (cb + 1) * 128]

        def emit_keymm(t):
            tp = t % 2
            if tp == 0:
                kp = pps_key.tile([TOK, 2 * H], F32, tag="key")
                key_pairs[t // 2] = [kp, None]
            kp = key_pairs[t // 2][0]
            out = kp[:, tp * H : (tp + 1) * H]
            if KNOBS["BIAS_FP8"]:
                nc.tensor.matmul(
                    out, bias8_l, bias8_r, start=True, stop=False,
                    perf_mode=PM.DoubleRow, skip_group_check=True,
                )
            else:
                nc.tensor.matmul(
                    out, onerow_b, bkrow, start=True, stop=False,
                    skip_group_check=True,
                )
            for c in range(HC):
                nc.tensor.matmul(
                    out,
                    lhsT_for(t, c),
                    wk_sb(c),
                    start=False,
                    stop=(c == HC - 1),
                    skip_group_check=True,
                )

        def emit_tanh(t):
            kp = key_pairs[t // 2][0]
            keys = pkeys.tile([TOK, 2 * H], BF16, tag="keys")
            nc.scalar.activation(keys, kp, AF.Tanh, bias=zero_sb)
            key_pairs[t // 2][1] = keys

        def emit_q():
            qacc_sb = pout.tile([128, HC * BPC], BF16, tag="qacc_sb")
            nc.vector.tensor_copy(qacc_sb, qacc_ps)
            q_ps = Td[0:BPC, :]
            for c in range(HC):
                nc.tensor.matmul(
                    q_ps,
                    qacc_sb[:, c * BPC : (c + 1) * BPC],
                    wq_sb(c),
                    start=(c == 0),
                    stop=(c == HC - 1),
                )
            q_sb = pout.tile([BPC, H], BF16, tag="q_sb")
            nc.vector.tensor_add(q_sb, q_ps, bq_sb)
            for which, i4t in (("qrep_nat", i4t_nat), ("qrep_perm", i4t_perm)):
                qr_ps = Td
                nc.tensor.matmul(qr_ps, i4t, q_sb, start=True, stop=True)
                qr_sb = pout.tile([128, H], BF16, tag=which)
                nc.vector.tensor_copy(qr_sb, qr_ps)
                state[which] = qr_sb
                del qr_ps, qr_sb
            # reuse the qacc bank: its group stopped and it was copied out
            numer_ps = Tq[:, 0 : HC * BPC]
            nc.tensor.matmul(
                numer_ps, onerow_b, zrow[:, 0:16], start=True, stop=False,
                skip_group_check=True,
            )
            state["numer_ps"] = numer_ps
            state["q_done"] = True
            if dbg:
                nc.sync.dma_start(aps["d_qacc"], qacc_sb)
                nc.sync.dma_start(aps["d_q"], q_sb)
                nc.sync.dma_start(aps["d_qrep"], state["qrep_nat"])

        def stage_prod(s):
            qrep = state["qrep_nat"]
            keys = key_pairs[s // 2][1]
            tp = s % 2
            if tp == 0:
                prod_pair = pprod.tile([TOK, 2 * H], BF16, tag="prod")
                sc_pair = psmall.tile([TOK, 2], F32, tag="sc")
                pair_bufs[s // 2] = (prod_pair, sc_pair)
            prod = pair_bufs[s // 2][0]
            kslice = keys[:, tp * H : (tp + 1) * H]
            pslice = prod[:, tp * H : (tp + 1) * H]
            a, b_ = KNOBS["PROD_DVE"]
            if s % b_ < a:
                nc.vector.tensor_mul(pslice, kslice, qrep)
            else:
                nc.gpsimd.tensor_mul(pslice, kslice, qrep)

        def stage_reduce(s):
            if s % 2 == 0:
                return
            prod, sc = pair_bufs[s // 2]
            ra, rb = KNOBS["RED_DVE"]
            if (s // 2) % rb < ra:
                nc.vector.tensor_reduce(
                    sc,
                    prod.rearrange("p (two i) -> p two i", two=2),
                    axis=mybir.AxisListType.X,
                    op=ALU.add,
                )
            else:
                nc.scalar.activation(
                    pc_tile, prod[:, 0:H], AF.Copy, accum_out=sc[:, 0:1]
                )
                nc.scalar.activation(
                    pc_tile, prod[:, H : 2 * H], AF.Copy, accum_out=sc[:, 1:2]
                )

        def stage_post(s):
            if s % 2 == 0:
                return
            sc = pair_bufs[s // 2][1]
            for tt in (0, 1):
                si = s - 1 + tt
                e_t = psmall.tile([TOK, 1], F32, tag="e")
                nc.scalar.activation(
                    e_t, sc[:, tt : tt + 1], AF.Exp,
                    bias=maskb_sb[:, si : si + 1],
                )
                if dbg:
                    nc.vector.tensor_copy(e_all[:, si : si + 1], e_t)
                ei_t = psmall.tile([TOK, BPC], BF16, tag="ei")
                nc.vector.tensor_scalar_mul(ei_t, i4_nat, e_t)
                ks = key_pairs[si // 2][1][:, tt * H : (tt + 1) * H]
                for c in range(HC):
                    nc.tensor.matmul(
                        state["numer_ps"][:, c * BPC : (c + 1) * BPC],
                        ks[:, c * 128 : (c + 1) * 128],
                        ei_t,
                        start=False,
                        stop=(si == NT - 1 and c == HC - 1),
                        skip_group_check=True,
                    )
                den_ps = state["den_ps"]
                if den_ps is None:
                    den_ps = Td[0:1, 0:BPC]
                    state["den_ps"] = den_ps
                nc.tensor.matmul(
                    den_ps, ones1, ei_t,
                    start=(si == 0), stop=(si == NT - 1),
                    skip_group_check=True,
                )

        def emit_score_stages(k):
            if not state["q_done"]:
                return
            while state["s_prod"] < min(NT, k - LAG + 1):
                stage_prod(state["s_prod"])
                state["s_prod"] += 1
            while state["s_red"] < min(NT, state["s_prod"] - 2):
                stage_reduce(state["s_red"])
                state["s_red"] += 1
            while state["s_post"] < min(NT, state["s_red"] - 2):
                stage_post(state["s_post"])
                state["s_post"] += 1
            if k >= NT + LAG:  # flush
                while state["s_red"] < NT:
                    stage_reduce(state["s_red"])
                    state["s_red"] += 1
                while state["s_post"] < NT:
                    stage_post(state["s_post"])
                    state["s_post"] += 1

        # ---------- emission schedule ----------
        for b in range(min(HB, NB)):
            emit_load(b)

        qsum_done = 0
        for k in range(NT):
            while (
                state["next_load"] < NB
                and k >= (state["next_load"] - HB) * LPD + LPD
            ):
                emit_load(state["next_load"])
                state["next_load"] += 1
            target = min(NT, ((k + 1) * NT + KQ - 1) // KQ)
            while qsum_done < target:
                emit_qsum(qsum_done)
                qsum_done += 1
            emit_score_stages(k)
            if k < NX:
                emit_transpose(k)
            elif k % 2 == 0:
                emit_xbar(k)
            emit_keymm(k)
            if k % 2 == 1:
                emit_tanh(k)
            if k == KQ:
                while qsum_done < NT:
                    emit_qsum(qsum_done)
                    qsum_done += 1
                emit_q()
        for k in range(NT, NT + LAG + 5):
            emit_score_stages(k)

        # ---------- epilogue ----------
        if dbg:
            num_dbg = pout.tile([128, HC * BPC], F32, tag="num_dbg")
            nc.vector.tensor_copy(num_dbg, state["numer_ps"])
            nc.sync.dma_start(aps["d_num"], num_dbg)
            nc.sync.dma_start(aps["d_keys0"], key_pairs[0][1])
            nc.sync.dma_start(aps["d_keys60"], key_pairs[60][1])
            nc.sync.dma_start(aps["d_e"], e_all)
            nc.sync.dma_start(aps["d_htx"], hTx_pairs[30])
        rcp_sb = pout.tile([1, BPC], F32, tag="rcp")
        nc.vector.reciprocal(rcp_sb, state["den_ps"])
        rr_ps = Td[:, 0:BPC]
        nc.tensor.matmul(rr_ps, onerow_f, rcp_sb, start=True, stop=True)
        if dbg:
            nc.sync.dma_start(aps["d_rcp"], rcp_sb)
        rr_sb = pout.tile([128, BPC], F32, tag="rr_sb")
        nc.vector.tensor_copy(rr_sb, rr_ps)
        out_sb = pout.tile([128, HC * BPC], F32, tag="out_sb")
        for c in range(HC):
            nc.vector.tensor_mul(
                out_sb[:, c * BPC : (c + 1) * BPC],
                state["numer_ps"][:, c * BPC : (c + 1) * BPC],
                rr_sb,
            )
        nc.sync.dma_start(y_num, out_sb)


_CACHE = {}


def _fix_dma_waits(nc):
    """walrus's DMA_DIRECT2D lowering has ONE sync-wait slot. The SWDGE h
    loads sit on one queue (qPoolDynamic): descriptor generation is program-
    ordered and same-buffer writes cannot reorder, so the WAW (DMA-lane) wait
    is hardware-redundant. Drop it; keep WAR/engine waits. Then sanity-check
    remaining wait counts (DMACopy: 1, others: 2, Drain/EVSEM exempt)."""
    for b in nc.m.functions[0].blocks:
        for i in b.instructions:
            si = i.sync_info
            if si is None:
                continue
            waits = list(si.on_wait)
            if (
                type(i).__name__ == "InstDMACopy"
                and getattr(i, "queue", "") == "qPoolDynamic"
                and len(waits) >= 2
            ):
                lane = [w for w in waits if w.ant_name.startswith("DMASW")]
                eng = [w for w in waits if not w.ant_name.startswith("DMA")]
                if len(lane) >= 1 and len(lane) + len(eng) == len(waits):
                    out0 = i.outs[0]
                    name = getattr(getattr(out0, "bass_ap", None), "tensor", None)
                    name = getattr(name, "name", "")
                    if name.startswith(("h", "slab")):
                        si.on_wait = eng
                        waits = eng
            if type(i).__name__ in ("InstDrain", "InstEventSemaphore"):
                continue
            limit = 1 if type(i).__name__ == "InstDMACopy" else 2
            if len(waits) > limit:
                raise RuntimeError(
                    f"{i.name} {type(i).__name__} has {len(waits)} waits "
                    f"(> {limit}): {[(w.ant_name, w.wait_value) for w in waits]}"
                )


def _get_program():
    if "nc" in _CACHE:
        return _CACHE["nc"], _CACHE["aps"]
    nc = bacc.Bacc(None, target_bir_lowering=False, debug=False)
    aps = {
        "x": nc.dram_tensor("x", [NT, TOK, H], F32, kind="ExternalInput").ap(),
        "packf": nc.dram_tensor("packf", [128, PACKF], F32, kind="ExternalInput").ap(),
        "packb": nc.dram_tensor("packb", [128, PACKB], BF16, kind="ExternalInput").ap(),
        "pack8": nc.dram_tensor("pack8", [1, PACK8], FP8, kind="ExternalInput").ap(),
        "y_num": nc.dram_tensor(
            "y_num", [128, HC * BPC], F32, kind="ExternalOutput"
        ).ap(),
    }
    if KNOBS["DEBUG"]:
        aps["d_qacc"] = nc.dram_tensor("d_qacc", [128, 16], BF16, kind="ExternalOutput").ap()
        aps["d_q"] = nc.dram_tensor("d_q", [BPC, H], BF16, kind="ExternalOutput").ap()
        aps["d_qrep"] = nc.dram_tensor("d_qrep", [128, H], BF16, kind="ExternalOutput").ap()
        aps["d_num"] = nc.dram_tensor("d_num", [128, 16], F32, kind="ExternalOutput").ap()
        aps["d_keys0"] = nc.dram_tensor("d_keys0", [128, 1024], BF16, kind="ExternalOutput").ap()
        aps["d_keys60"] = nc.dram_tensor("d_keys60", [128, 1024], BF16, kind="ExternalOutput").ap()
        aps["d_rcp"] = nc.dram_tensor("d_rcp", [1, BPC], F32, kind="ExternalOutput").ap()
        aps["d_e"] = nc.dram_tensor("d_e", [TOK, NT], F32, kind="ExternalOutput").ap()
        aps["d_htx"] = nc.dram_tensor("d_htx", [128, 2 * H], BF16, kind="ExternalOutput").ap()
    with tile.TileContext(nc) as tc:
        _build_kernel_body(tc, aps)
    nc.finalize()
    _fix_dma_waits(nc)
    _CACHE["nc"] = nc
    _CACHE["aps"] = aps
    return nc, aps


def _make_in_maps(hidden_states, Wq, bq, Wk, bk, lengths):
    hidden = np.asarray(hidden_states, dtype=np.float32)
    Wq = np.asarray(Wq, dtype=np.float32)
    Wk = np.asarray(Wk, dtype=np.float32)
    bqv = np.asarray(bq, dtype=np.float32)
    bkv = np.asarray(bk, dtype=np.float32)
    lens = np.asarray(lengths).astype(np.int64)

    NX = KNOBS["NX"]
    p = np.arange(128)
    perm = _xbar_perm()

    packb = np.zeros((128, PACKB), dtype=BF16NP)
    packb[:, OFFB_WK : OFFB_WK + 2048] = (
        np.ascontiguousarray(Wk.T)
        .reshape(HC, 128, H)
        .transpose(1, 0, 2)
        .reshape(128, 2048)
        .astype(BF16NP)
    )
    packb[:, OFFB_WQ : OFFB_WQ + 2048] = (
        (np.ascontiguousarray(Wq.T) / S)
        .reshape(HC, 128, H)
        .transpose(1, 0, 2)
        .reshape(128, 2048)
        .astype(BF16NP)
    )
    packb[:, OFFB_ID : OFFB_ID + 128] = np.eye(128, dtype=BF16NP)
    packb[:, OFFB_I4_NAT : OFFB_I4_NAT + BPC] = (
        p[:, None] % BPC == np.arange(BPC)[None, :]
    ).astype(BF16NP)
    packb[:, OFFB_I4_PERM : OFFB_I4_PERM + BPC] = (
        perm[:, None] % BPC == np.arange(BPC)[None, :]
    ).astype(BF16NP)
    packb[:, OFFB_ONES1] = BF16NP(1.0)
    packb[0, OFFB_BKROW : OFFB_BKROW + H] = bkv.astype(BF16NP)
    packb[0, OFFB_ONEROW : OFFB_ONEROW + 128] = BF16NP(1.0)
    packb[0:BPC, OFFB_I4T_NAT : OFFB_I4T_NAT + 128] = (
        p[None, :] % BPC == np.arange(BPC)[:, None]
    ).astype(BF16NP)
    packb[0:BPC, OFFB_I4T_PERM : OFFB_I4T_PERM + 128] = (
        perm[None, :] % BPC == np.arange(BPC)[:, None]
    ).astype(BF16NP)

    pack8 = np.zeros((1, PACK8), dtype=FP8NP)
    pack8[0, OFF8_L : OFF8_L + 128] = FP8NP(1.0)
    pack8[0, OFF8_R : OFF8_R + H] = bkv.astype(FP8NP)

    base_packf = np.zeros((128, PACKF), dtype=np.float32)
    base_packf[0:BPC, OFF_BQ : OFF_BQ + H] = bqv[None, :]
    base_packf[0, OFF_ONEROW_F : OFF_ONEROW_F + 128] = 1.0

    in_maps = []
    t_idx = np.arange(NT)
    for core in range(NCORES):
        xc = np.ascontiguousarray(
            hidden[:, core * BPC : (core + 1) * BPC, :]
        ).reshape(NT, TOK, H)
        packf = base_packf.copy()
        tok_of_p = np.broadcast_to(p[:, None], (128, NT))
        b_of_p = core * BPC + tok_of_p % BPC
        s_full = SS * t_idx[None, :] + tok_of_p // BPC
        valid = s_full < lens[b_of_p]
        packf[:, OFF_MASK : OFF_MASK + NT] = np.where(valid, 0.0, MASK_NEG)
        in_maps.append({"x": xc, "packf": packf, "packb": packb, "pack8": pack8})
    return in_maps


def run(hidden_states, Wq, bq, Wk, bk, lengths, trace=False):
    """Run on 8 cores; returns (output [B, H] fp32, BassKernelResults)."""
    nc, _ = _get_program()
    in_maps = _make_in_maps(hidden_states, Wq, bq, Wk, bk, lengths)
    res = run_bass_kernel_spmd(
        nc, in_maps, core_ids=list(range(NCORES)), trace=trace
    )
    outs = []
    for r in res.results:
        ynum = np.asarray(r["y_num"])  # [128 i_local, (c, g)], already / den
        o = ynum.reshape(128, HC, BPC).transpose(2, 1, 0).reshape(BPC, H)
        outs.append(o)
    out = np.concatenate(outs, axis=0)
    return out.astype(np.float32), res


def kernel(hidden_states, Wq, bq, Wk, bk, lengths):
    out, _ = run(hidden_states, Wq, bq, Wk, bk, lengths)
    return out


# revision 25
# speedup vs baseline: 1.1067x; 1.1067x over previous
"""Trainium2 Bass kernel for nn_Attention_82660940579436.

Computation (see reference):
    q     = mean_s(hidden @ Wq.T + bq)            [B, H]
    key   = tanh(hidden @ Wk.T + bk)              [S, B, H]
    score = einsum('bsh,bh->bs', key, q) + mask   [B, S]
    out   = softmax(score) @ key                  [B, H]

Sharding: data-parallel over batch. B=32 over 8 cores -> 4 batches/core.

Fused streaming design (single pass, software-pipelined emission):
  - h tiles [128 tok=(s,g), 512 j] stream in via SWDGE (fp32->bf16 cast),
    8 tiles per DMA, 12-slab ring.
  - q-sum runs as tiny rank-4 PE matmuls on the UNtransposed h tiles
    (out [128 j, (c,g)] accumulated in PSUM over all tiles) -- q is ready
    right after the last load (~48us), not after the last key matmul.
  - hT: first NX tiles via PE transpose + DVE copy; the rest via the DMA
    XBAR transpose (SBUF->SBUF, 2 tiles per instruction). The xbar emits
    a fixed token permutation (position m holds token 2*(m%64)+m//64 of
    its tile); the permutation is absorbed into the constant packs (mask
    columns, ind4, qrep) -- every downstream op is token-parallel or a
    permutation-invariant contraction over tokens.
  - key matmul: bf16, moving operand = Wk chunks (4 x 512 cols); bias via
    a single fp8e4 DoubleRow matmul (K=1, 0.5 cycles/col, exact for these
    operands -- verified on HW).
  - tanh on ACT over 2-tile PSUM batches -> keys ring.
  - score pipeline trails by LAG tiles (the q barrier): prod on DVE/Pool,
    row-reduce on DVE (2-tile) / ACT (copy+accum), exp with the mask as a
    per-partition bias, ei = ind4*e (DVE TSP), numer/den as rank-4/1 PE
    matmuls accumulated in PSUM ([128 i, (c,g)] resp. [1, 4]).
  - epilogue: rcp(den), replicate via tiny matmul, scale, DMA out
    [128, (c,g)]; host reorders to [B, H].

fp8 for the big matmul was measured (numpy) at rel_err 3.9e-2 -- over the
2e-2 gate -- so the key matmul stays bf16.
"""

import sys
from contextlib import ExitStack

import numpy as np

if "/opt/trn_rl_repo" not in sys.path:
    sys.path.insert(0, "/opt/trn_rl_repo")

import ml_dtypes  # noqa: E402

import concourse.bacc as bacc  # noqa: E402
import concourse.mybir as mybir  # noqa: E402
import concourse.tile as tile  # noqa: E402
from concourse.bass_utils import run_bass_kernel_spmd  # noqa: E402

S, B, H = 4096, 32, 512
NCORES = 8
BPC = B // NCORES  # 4 batches per core
NT = 128  # tiles per core
SS = S // NT  # 32 s-positions per tile
TOK = SS * BPC  # 128 tokens per tile
HC = H // 128  # 4 chunks of the H (j / i) dims
MASK_NEG = -60.0
F32 = mybir.dt.float32
BF16 = mybir.dt.bfloat16
FP8 = mybir.dt.float8e4
AF = mybir.ActivationFunctionType
ALU = mybir.AluOpType
PM = mybir.MatmulPerfMode
BF16NP = ml_dtypes.bfloat16
FP8NP = ml_dtypes.float8_e4m3

# ---- tuning knobs ----
KNOBS = {
    "NX": 40,  # tiles using PE transpose; rest use xbar DMA transpose
    "LAG": 44,  # score pipeline trails key pipeline by this many tiles
    "KQ": 45,  # emit q computation after this key tile
    "H_BUFS": 24,  # h ring slabs (LOADS_PER_DMA tiles each)
    "K_BUFS": 28,  # keys ring (2 tiles each)
    "HT_BUFS": 3,  # hT sbuf bufs (PE-transposed tiles)
    "HTX_BUFS": 4,  # hT sbuf bufs (xbar pairs)
    "PROD_DVE": (11, 20),  # prod on DVE for t%20 < 11, else gpsimd
    "RED_DVE": (7, 10),  # reduce on DVE for pair%10 < 7, else ACT
    "BIAS_FP8": False,  # DR matmuls corrupt interleaved PSUM accumulation on HW
    "LOADS_PER_DMA": 4,
    "DEBUG": False,
}

# fp32 const pack layout (offsets in fp32 elements, [128, PACKF] tensor)
OFF_MASK = 0  # [128, NT] mask bias (0 / MASK_NEG), col=tile (perm-aware)
OFF_BQ = NT  # [4, 512] bq rows
OFF_ONEROW_F = NT + 512  # [1, 128] ones (fp32) for rcp_rep matmul
OFF_ZERO = NT + 640  # [128, 1] zeros (tanh bias)
PACKF = NT + 641

# bf16 const pack layout ([128, PACKB])
OFFB_WK = 0  # [128, 2048] WkT chunks
OFFB_WQ = 2048  # [128, 2048] (WqT/S) chunks
OFFB_ID = 4096  # [128, 128] identity
OFFB_I4_NAT = 4224  # [128, 4] indicator
OFFB_I4_PERM = 4228  # [128, 4] indicator (xbar permuted)
OFFB_ONES1 = 4232  # [128, 1] ones
OFFB_ZROW = 4233  # [1, 23] zeros (zero-init matmuls)
OFFB_BKROW = 4256  # [1, 512] bk (bf16 bias fallback)
OFFB_ONEROW = 4768  # [1, 128] ones row
OFFB_I4T_NAT = 4896  # [4, 128] indicator transposed (bf16)
OFFB_I4T_PERM = 5024  # [4, 128] indicator transposed, permuted (bf16)
PACKB = 5152

# fp8 pack ([1, PACK8]): DoubleRow bias operands
OFF8_L = 0  # [1, 256] lhsT pairs: slot0 = ones(128), slot1 = zeros
OFF8_R = 256  # [1, 1024] rhs pairs: slot0 = bk, slot1 = zeros
PACK8 = 1280


def _xbar_perm():
    """Token permutation of the xbar output: position m holds token 2*(m%64)+m//64."""
    m = np.arange(128)
    return 2 * (m % 64) + m // 64


def _build_kernel_body(tc, aps):
    nc = tc.nc
    x, packf, packb, pack8 = aps["x"], aps["packf"], aps["packb"], aps["pack8"]
    y_num = aps["y_num"]
    dbg = KNOBS["DEBUG"]

    NX = KNOBS["NX"]
    LAG = KNOBS["LAG"]
    KQ = KNOBS["KQ"]
    LPD = KNOBS["LOADS_PER_DMA"]
    NB = NT // LPD  # load batches
    HB = KNOBS["H_BUFS"]

    with ExitStack() as ctx:
        consts = ctx.enter_context(tc.tile_pool(name="consts", bufs=1))
        ph = ctx.enter_context(tc.tile_pool(name="h", bufs=HB))
        phT = ctx.enter_context(tc.tile_pool(name="hT", bufs=KNOBS["HT_BUFS"]))
        phTx = ctx.enter_context(tc.tile_pool(name="hTx", bufs=KNOBS["HTX_BUFS"]))
        pkeys = ctx.enter_context(tc.tile_pool(name="keys", bufs=KNOBS["K_BUFS"]))
        pprod = ctx.enter_context(tc.tile_pool(name="prod", bufs=4))
        psmall = ctx.enter_context(tc.tile_pool(name="small", bufs=6))
        pout = ctx.enter_context(tc.tile_pool(name="out", bufs=1))
        pps_q = ctx.enter_context(tc.tile_pool(name="ps_q", bufs=1, space="PSUM"))
        pps_key = ctx.enter_context(tc.tile_pool(name="ps_key", bufs=2, space="PSUM"))
        pps_hT = ctx.enter_context(tc.tile_pool(name="ps_hT", bufs=2, space="PSUM"))
        pps_d = ctx.enter_context(tc.tile_pool(name="ps_d", bufs=1, space="PSUM"))

        # ---- constants: one DMA per pack ----
        cf = consts.tile([128, PACKF], F32)
        nc.sync.dma_start(cf, packf)
        cb = consts.tile([128, PACKB], BF16)
        nc.sync.dma_start(cb, packb)
        c8 = consts.tile([1, PACK8], FP8)
        nc.sync.dma_start(c8, pack8)

        def wk_sb(c):
            return cb[:, OFFB_WK + c * 512 : OFFB_WK + (c + 1) * 512]

        def wq_sb(c):
            return cb[:, OFFB_WQ + c * 512 : OFFB_WQ + (c + 1) * 512]

        id_sb = cb[:, OFFB_ID : OFFB_ID + 128]
        maskb_sb = cf[:, OFF_MASK : OFF_MASK + NT]
        bq_sb = cf[0:BPC, OFF_BQ : OFF_BQ + H]
        onerow_f = cf[0:1, OFF_ONEROW_F : OFF_ONEROW_F + 128]
        zero_sb = cf[:, OFF_ZERO : OFF_ZERO + 1]
        i4_nat = cb[:, OFFB_I4_NAT : OFFB_I4_NAT + BPC]
        i4_perm = cb[:, OFFB_I4_PERM : OFFB_I4_PERM + BPC]
        ones1 = cb[:, OFFB_ONES1 : OFFB_ONES1 + 1]
        zrow = cb[0:1, OFFB_ZROW : OFFB_ZROW + 23]
        bkrow = cb[0:1, OFFB_BKROW : OFFB_BKROW + H]
        onerow_b = cb[0:1, OFFB_ONEROW : OFFB_ONEROW + 128]
        i4t_nat = cb[0:BPC, OFFB_I4T_NAT : OFFB_I4T_NAT + 128]
        i4t_perm = cb[0:BPC, OFFB_I4T_PERM : OFFB_I4T_PERM + 128]
        bias8_l = c8[:, OFF8_L : OFF8_L + 256].rearrange(
            "p (two m) -> p two m", two=2
        )
        bias8_r = c8[:, OFF8_R : OFF8_R + 1024].rearrange(
            "p (two n) -> p two n", two=2
        )

        # Shared PSUM banks: Tq (dummies then qacc), Td (q/qrep/den/rr chain).
        Tq = pps_q.tile([128, 512], F32, tag="qa")
        Td = pps_d.tile([128, 512], F32, tag="d")

        # Dummy PE ops observing each const-pack DMA lane once (walrus allows
        # only ONE sync-wait per Matmult).
        nc.tensor.matmul(
            Tq, bias8_l, bias8_r, start=True, stop=True, perf_mode=PM.DoubleRow
        )
        nc.tensor.matmul(
            Tq[:, 0:128], onerow_b, onerow_b, start=True, stop=True
        )
        nc.tensor.matmul(
            Tq[:, 128:256], onerow_f, cf[0:1, 0:128], start=True, stop=True
        )

        # qacc: [128 j_local, (c, g)] accumulated over all tiles; zero-init
        # matmul so per-tile qsum matmuls never need start=True.
        qacc_ps = Tq[:, 0 : HC * BPC]
        nc.tensor.matmul(
            qacc_ps, onerow_b, zrow[:, 0:16], start=True, stop=False,
            skip_group_check=True,
        )

        h_slabs = [None] * HB
        hT_nat = [None] * NT  # per-tile [128, 512] (PE transpose path)
        hTx_pairs = [None] * (NT // 2)  # per-pair [128, 1024] (xbar path)
        key_pairs = [None] * (NT // 2)
        pc_tile = pout.tile([TOK, H], BF16, tag="pc")  # ACT-reduce dump
        pair_bufs = {}
        if dbg:
            e_all = pout.tile([TOK, NT], F32, tag="e_all")
        else:
            e_all = None
        state = {
            "q_done": False,
            "qrep_nat": None,
            "qrep_perm": None,
            "numer_ps": None,
            "den_ps": None,
            "prod_pair": None,
            "sc_pair": None,
            "next_load": min(HB, NB),
            "s_prod": 0,
            "s_red": 0,
            "s_post": 0,
        }

        def h_tile(t):
            return h_slabs[(t // LPD) % HB][:, (t % LPD) * H : (t % LPD + 1) * H]

        def emit_load(b):
            slab = ph.tile([TOK, LPD * H], BF16, tag="h")
            h_slabs[b % HB] = slab
            nc.gpsimd.dma_start(
                slab, x[b * LPD : (b + 1) * LPD].rearrange("t p j -> p t j")
            )

        def emit_qsum(t):
            ht = h_tile(t)
            for c in range(HC):
                nc.tensor.matmul(
                    qacc_ps[:, c * BPC : (c + 1) * BPC],
                    ht[:, c * 128 : (c + 1) * 128],
                    i4_nat,
                    start=False,
                    stop=(t == NT - 1 and c == HC - 1),
                    skip_group_check=True,
                )

        def emit_transpose(t):
            hT_ps = pps_hT.tile([128, H], BF16, tag="hT")
            ht = h_tile(t)
            for c in range(HC):
                nc.tensor.transpose(
                    hT_ps[:, c * 128 : (c + 1) * 128],
                    ht[:, c * 128 : (c + 1) * 128],
                    id_sb,
                )
            hT_sb = phT.tile([128, H], BF16, tag="hT_sb")
            nc.vector.tensor_copy(hT_sb, hT_ps)
            hT_nat[t] = hT_sb

        def emit_xbar(t):
            # XBAR transpose of the pair (t, t+1). With a 3D out AP
            # [p, cb (stride 128), k (stride 1)] the xbar lands NATURALLY:
            # hTx[j, cb*128 + tok] = h[tok, cb*128 + j], cb = tp*4 + c.
            slab = h_slabs[(t // LPD) % HB]
            lo = (t % LPD) * H
            hTx = phTx.tile([128, 2 * H], BF16, tag="hTx")
            nc.sync.dma_start(
                hTx.rearrange("p (cb k) -> p cb k", k=128),
                slab[:, lo : lo + 2 * H],
                transpose=True,
            )
            hTx_pairs[t // 2] = hTx

        def lhsT_for(t, c):
            if t < NX:
                return hT_nat[t][:, c * 128 : (c + 1) * 128]
            hTx = hTx_pairs[t // 2]
            cb = (t % 2) * HC + c
            return hTx[:, cb * 128 : (cb + 1) * 128]

        def emit_keymm(t):
            tp = t % 2
            if tp == 0:
                kp = pps_key.tile([TOK, 2 * H], F32, tag="key")
                key_pairs[t // 2] = [kp, None]
            kp = key_pairs[t // 2][0]
            out = kp[:, tp * H : (tp + 1) * H]
            if KNOBS["BIAS_FP8"]:
                nc.tensor.matmul(
                    out, bias8_l, bias8_r, start=True, stop=False,
                    perf_mode=PM.DoubleRow, skip_group_check=True,
                )
            else:
                nc.tensor.matmul(
                    out, onerow_b, bkrow, start=True, stop=False,
                    skip_group_check=True,
                )
            for c in range(HC):
                nc.tensor.matmul(
                    out,
                    lhsT_for(t, c),
                    wk_sb(c),
                    start=False,
                    stop=(c == HC - 1),
                    skip_group_check=True,
                )

        def emit_tanh(t):
            kp = key_pairs[t // 2][0]
            keys = pkeys.tile([TOK, 2 * H], BF16, tag="keys")
            nc.scalar.activation(keys, kp, AF.Tanh, bias=zero_sb)
            key_pairs[t // 2][1] = keys

        def emit_q():
            qacc_sb = pout.tile([128, HC * BPC], BF16, tag="qacc_sb")
            nc.vector.tensor_copy(qacc_sb, qacc_ps)
            q_ps = Td[0:BPC, :]
            for c in range(HC):
                nc.tensor.matmul(
                    q_ps,
                    qacc_sb[:, c * BPC : (c + 1) * BPC],
                    wq_sb(c),
                    start=(c == 0),
                    stop=(c == HC - 1),
                )
            q_sb = pout.tile([BPC, H], BF16, tag="q_sb")
            nc.vector.tensor_add(q_sb, q_ps, bq_sb)
            for which, i4t in (("qrep_nat", i4t_nat), ("qrep_perm", i4t_perm)):
                qr_ps = Td
                nc.tensor.matmul(qr_ps, i4t, q_sb, start=True, stop=True)
                qr_sb = pout.tile([128, H], BF16, tag=which)
                nc.vector.tensor_copy(qr_sb, qr_ps)
                state[which] = qr_sb
                del qr_ps, qr_sb
            # reuse the qacc bank: its group stopped and it was copied out
            numer_ps = Tq[:, 0 : HC * BPC]
            nc.tensor.matmul(
                numer_ps, onerow_b, zrow[:, 0:16], start=True, stop=False,
                skip_group_check=True,
            )
            state["numer_ps"] = numer_ps
            state["q_done"] = True
            if dbg:
                nc.sync.dma_start(aps["d_qacc"], qacc_sb)
                nc.sync.dma_start(aps["d_q"], q_sb)
                nc.sync.dma_start(aps["d_qrep"], state["qrep_nat"])

        def stage_prod(s):
            qrep = state["qrep_nat"]
            keys = key_pairs[s // 2][1]
            tp = s % 2
            if tp == 0:
                prod_pair = pprod.tile([TOK, 2 * H], BF16, tag="prod")
                sc_pair = psmall.tile([TOK, 2], F32, tag="sc")
                pair_bufs[s // 2] = (prod_pair, sc_pair)
            prod = pair_bufs[s // 2][0]
            kslice = keys[:, tp * H : (tp + 1) * H]
            pslice = prod[:, tp * H : (tp + 1) * H]
            a, b_ = KNOBS["PROD_DVE"]
            if s % b_ < a:
                nc.vector.tensor_mul(pslice, kslice, qrep)
            else:
                nc.gpsimd.tensor_mul(pslice, kslice, qrep)

        def stage_reduce(s):
            if s % 2 == 0:
                return
            prod, sc = pair_bufs[s // 2]
            ra, rb = KNOBS["RED_DVE"]
            if (s // 2) % rb < ra:
                nc.vector.tensor_reduce(
                    sc,
                    prod.rearrange("p (two i) -> p two i", two=2),
                    axis=mybir.AxisListType.X,
                    op=ALU.add,
                )
            else:
                nc.scalar.activation(
                    pc_tile, prod[:, 0:H], AF.Copy, accum_out=sc[:, 0:1]
                )
                nc.scalar.activation(
                    pc_tile, prod[:, H : 2 * H], AF.Copy, accum_out=sc[:, 1:2]
                )

        def stage_post(s):
            if s % 2 == 0:
                return
            sc = pair_bufs[s // 2][1]
            for tt in (0, 1):
                si = s - 1 + tt
                e_t = psmall.tile([TOK, 1], F32, tag="e")
                nc.scalar.activation(
                    e_t, sc[:, tt : tt + 1], AF.Exp,
                    bias=maskb_sb[:, si : si + 1],
                )
                if dbg:
                    nc.vector.tensor_copy(e_all[:, si : si + 1], e_t)
                ei_t = psmall.tile([TOK, BPC], BF16, tag="ei")
                nc.vector.tensor_scalar_mul(ei_t, i4_nat, e_t)
                ks = key_pairs[si // 2][1][:, tt * H : (tt + 1) * H]
                for c in range(HC):
                    nc.tensor.matmul(
                        state["numer_ps"][:, c * BPC : (c + 1) * BPC],
                        ks[:, c * 128 : (c + 1) * 128],
                        ei_t,
                        start=False,
                        stop=(si == NT - 1 and c == HC - 1),
                        skip_group_check=True,
                    )
                den_ps = state["den_ps"]
                if den_ps is None:
                    den_ps = Td[0:1, 0:BPC]
                    state["den_ps"] = den_ps
                nc.tensor.matmul(
                    den_ps, ones1, ei_t,
                    start=(si == 0), stop=(si == NT - 1),
                    skip_group_check=True,
                )

        def emit_score_stages(k):
            if not state["q_done"]:
                return
            while state["s_prod"] < min(NT, k - LAG + 1):
                stage_prod(state["s_prod"])
                state["s_prod"] += 1
            while state["s_red"] < min(NT, state["s_prod"] - 2):
                stage_reduce(state["s_red"])
                state["s_red"] += 1
            while state["s_post"] < min(NT, state["s_red"] - 2):
                stage_post(state["s_post"])
                state["s_post"] += 1
            if k >= NT + LAG:  # flush
                while state["s_red"] < NT:
                    stage_reduce(state["s_red"])
                    state["s_red"] += 1
                while state["s_post"] < NT:
                    stage_post(state["s_post"])
                    state["s_post"] += 1

        # ---------- emission schedule ----------
        for b in range(min(HB, NB)):
            emit_load(b)

        qsum_done = 0
        for k in range(NT):
            while (
                state["next_load"] < NB
                and k >= (state["next_load"] - HB) * LPD + LPD
            ):
                emit_load(state["next_load"])
                state["next_load"] += 1
            target = min(NT, ((k + 1) * NT + KQ - 1) // KQ)
            while qsum_done < target:
                emit_qsum(qsum_done)
                qsum_done += 1
            emit_score_stages(k)
            if k < NX:
                emit_transpose(k)
            elif k % 2 == 0:
                emit_xbar(k)
            emit_keymm(k)
            if k % 2 == 1:
                emit_tanh(k)
            if k == KQ:
                while qsum_done < NT:
                    emit_qsum(qsum_done)
                    qsum_done += 1
                emit_q()
        for k in range(NT, NT + LAG + 5):
            emit_score_stages(k)

        # ---------- epilogue ----------
        if dbg:
            num_dbg = pout.tile([128, HC * BPC], F32, tag="num_dbg")
            nc.vector.tensor_copy(num_dbg, state["numer_ps"])
            nc.sync.dma_start(aps["d_num"], num_dbg)
            nc.sync.dma_start(aps["d_keys0"], key_pairs[0][1])
            nc.sync.dma_start(aps["d_keys60"], key_pairs[60][1])
            nc.sync.dma_start(aps["d_e"], e_all)
            nc.sync.dma_start(aps["d_htx"], hTx_pairs[30])
        rcp_sb = pout.tile([1, BPC], F32, tag="rcp")
        nc.vector.reciprocal(rcp_sb, state["den_ps"])
        rr_ps = Td[:, 0:BPC]
        nc.tensor.matmul(rr_ps, onerow_f, rcp_sb, start=True, stop=True)
        if dbg:
            nc.sync.dma_start(aps["d_rcp"], rcp_sb)
        rr_sb = pout.tile([128, BPC], F32, tag="rr_sb")
        nc.vector.tensor_copy(rr_sb, rr_ps)
        out_sb = pout.tile([128, HC * BPC], F32, tag="out_sb")
        for c in range(HC):
            nc.vector.tensor_mul(
                out_sb[:, c * BPC : (c + 1) * BPC],
                state["numer_ps"][:, c * BPC : (c + 1) * BPC],
                rr_sb,
            )
        nc.sync.dma_start(y_num, out_sb)


_CACHE = {}


def _fix_dma_waits(nc):
    """walrus's DMA_DIRECT2D lowering has ONE sync-wait slot. The SWDGE h
    loads sit on one queue (qPoolDynamic): descriptor generation is program-
    ordered and same-buffer writes cannot reorder, so the WAW (DMA-lane) wait
    is hardware-redundant. Drop it; keep WAR/engine waits. Then sanity-check
    remaining wait counts (DMACopy: 1, others: 2, Drain/EVSEM exempt)."""
    for b in nc.m.functions[0].blocks:
        for i in b.instructions:
            si = i.sync_info
            if si is None:
                continue
            waits = list(si.on_wait)
            if (
                type(i).__name__ == "InstDMACopy"
                and getattr(i, "queue", "") == "qPoolDynamic"
                and len(waits) >= 2
            ):
                lane = [w for w in waits if w.ant_name.startswith("DMASW")]
                eng = [w for w in waits if not w.ant_name.startswith("DMA")]
                if len(lane) >= 1 and len(lane) + len(eng) == len(waits):
                    out0 = i.outs[0]
                    name = getattr(getattr(out0, "bass_ap", None), "tensor", None)
                    name = getattr(name, "name", "")
                    if name.startswith(("h", "slab")):
                        si.on_wait = eng
                        waits = eng
            if type(i).__name__ in ("InstDrain", "InstEventSemaphore"):
                continue
            limit = 1 if type(i).__name__ == "InstDMACopy" else 2
            if len(waits) > limit:
                raise RuntimeError(
                    f"{i.name} {type(i).__name__} has {len(waits)} waits "
                    f"(> {limit}): {[(w.ant_name, w.wait_value) for w in waits]}"
                )


def _get_program():
    if "nc" in _CACHE:
        return _CACHE["nc"], _CACHE["aps"]
    nc = bacc.Bacc(None, target_bir_lowering=False, debug=False)
    aps = {
        "x": nc.dram_tensor("x", [NT, TOK, H], F32, kind="ExternalInput").ap(),
        "packf": nc.dram_tensor("packf", [128, PACKF], F32, kind="ExternalInput").ap(),
        "packb": nc.dram_tensor("packb", [128, PACKB], BF16, kind="ExternalInput").ap(),
        "pack8": nc.dram_tensor("pack8", [1, PACK8], FP8, kind="ExternalInput").ap(),
        "y_num": nc.dram_tensor(
            "y_num", [128, HC * BPC], F32, kind="ExternalOutput"
        ).ap(),
    }
    if KNOBS["DEBUG"]:
        aps["d_qacc"] = nc.dram_tensor("d_qacc", [128, 16], BF16, kind="ExternalOutput").ap()
        aps["d_q"] = nc.dram_tensor("d_q", [BPC, H], BF16, kind="ExternalOutput").ap()
        aps["d_qrep"] = nc.dram_tensor("d_qrep", [128, H], BF16, kind="ExternalOutput").ap()
        aps["d_num"] = nc.dram_tensor("d_num", [128, 16], F32, kind="ExternalOutput").ap()
        aps["d_keys0"] = nc.dram_tensor("d_keys0", [128, 1024], BF16, kind="ExternalOutput").ap()
        aps["d_keys60"] = nc.dram_tensor("d_keys60", [128, 1024], BF16, kind="ExternalOutput").ap()
        aps["d_rcp"] = nc.dram_tensor("d_rcp", [1, BPC], F32, kind="ExternalOutput").ap()
        aps["d_e"] = nc.dram_tensor("d_e", [TOK, NT], F32, kind="ExternalOutput").ap()
        aps["d_htx"] = nc.dram_tensor("d_htx", [128, 2 * H], BF16, kind="ExternalOutput").ap()
    with tile.TileContext(nc) as tc:
        _build_kernel_body(tc, aps)
    nc.finalize()
    _fix_dma_waits(nc)
    _CACHE["nc"] = nc
    _CACHE["aps"] = aps
    return nc, aps


def _make_in_maps(hidden_states, Wq, bq, Wk, bk, lengths):
    hidden = np.asarray(hidden_states, dtype=np.float32)
    Wq = np.asarray(Wq, dtype=np.float32)
    Wk = np.asarray(Wk, dtype=np.float32)
    bqv = np.asarray(bq, dtype=np.float32)
    bkv = np.asarray(bk, dtype=np.float32)
    lens = np.asarray(lengths).astype(np.int64)

    NX = KNOBS["NX"]
    p = np.arange(128)
    perm = _xbar_perm()

    packb = np.zeros((128, PACKB), dtype=BF16NP)
    packb[:, OFFB_WK : OFFB_WK + 2048] = (
        np.ascontiguousarray(Wk.T)
        .reshape(HC, 128, H)
        .transpose(1, 0, 2)
        .reshape(128, 2048)
        .astype(BF16NP)
    )
    packb[:, OFFB_WQ : OFFB_WQ + 2048] = (
        (np.ascontiguousarray(Wq.T) / S)
        .reshape(HC, 128, H)
        .transpose(1, 0, 2)
        .reshape(128, 2048)
        .astype(BF16NP)
    )
    packb[:, OFFB_ID : OFFB_ID + 128] = np.eye(128, dtype=BF16NP)
    packb[:, OFFB_I4_NAT : OFFB_I4_NAT + BPC] = (
        p[:, None] % BPC == np.arange(BPC)[None, :]
    ).astype(BF16NP)
    packb[:, OFFB_I4_PERM : OFFB_I4_PERM + BPC] = (
        perm[:, None] % BPC == np.arange(BPC)[None, :]
    ).astype(BF16NP)
    packb[:, OFFB_ONES1] = BF16NP(1.0)
    packb[0, OFFB_BKROW : OFFB_BKROW + H] = bkv.astype(BF16NP)
    packb[0, OFFB_ONEROW : OFFB_ONEROW + 128] = BF16NP(1.0)
    packb[0:BPC, OFFB_I4T_NAT : OFFB_I4T_NAT + 128] = (
        p[None, :] % BPC == np.arange(BPC)[:, None]
    ).astype(BF16NP)
    packb[0:BPC, OFFB_I4T_PERM : OFFB_I4T_PERM + 128] = (
        perm[None, :] % BPC == np.arange(BPC)[:, None]
    ).astype(BF16NP)

    pack8 = np.zeros((1, PACK8), dtype=FP8NP)
    pack8[0, OFF8_L : OFF8_L + 128] = FP8NP(1.0)
    pack8[0, OFF8_R : OFF8_R + H] = bkv.astype(FP8NP)

    base_packf = np.zeros((128, PACKF), dtype=np.float32)
    base_packf[0:BPC, OFF_BQ : OFF_BQ + H] = bqv[None, :]
    base_packf[0, OFF_ONEROW_F : OFF_ONEROW_F + 128] = 1.0

    in_maps = []
    t_idx = np.arange(NT)
    for core in range(NCORES):
        xc = np.ascontiguousarray(
            hidden[:, core * BPC : (core + 1) * BPC, :]
        ).reshape(NT, TOK, H)
        packf = base_packf.copy()
        tok_of_p = np.broadcast_to(p[:, None], (128, NT))
        b_of_p = core * BPC + tok_of_p % BPC
        s_full = SS * t_idx[None, :] + tok_of_p // BPC
        valid = s_full < lens[b_of_p]
        packf[:, OFF_MASK : OFF_MASK + NT] = np.where(valid, 0.0, MASK_NEG)
        in_maps.append({"x": xc, "packf": packf, "packb": packb, "pack8": pack8})
    return in_maps


def run(hidden_states, Wq, bq, Wk, bk, lengths, trace=False):
    """Run on 8 cores; returns (output [B, H] fp32, BassKernelResults)."""
    nc, _ = _get_program()
    in_maps = _make_in_maps(hidden_states, Wq, bq, Wk, bk, lengths)
    res = run_bass_kernel_spmd(
        nc, in_maps, core_ids=list(range(NCORES)), trace=trace
    )
    outs = []
    for r in res.results:
        ynum = np.asarray(r["y_num"])  # [128 i_local, (c, g)], already / den
        o = ynum.reshape(128, HC, BPC).transpose(2, 1, 0).reshape(BPC, H)
        outs.append(o)
    out = np.concatenate(outs, axis=0)
    return out.astype(np.float32), res


def kernel(hidden_states, Wq, bq, Wk, bk, lengths):
    out, _ = run(hidden_states, Wq, bq, Wk, bk, lengths)
    return out
